# revision 1
# baseline (speedup 1.0000x reference)
"""Trainium2 Bass kernel for the NP/NY/NU RNN scan (nn_BlackBoxModel_24489903521937).

Model (per step t, batch row b):
    x_t   = [y_t, y_{t-4..t-1}, u_{t-4..t-1}, u_t]          (60)
    h1    = tanh(x_t @ W1 + b1)                              (128)
    h2    = tanh(h1 @ W2 + b2)                               (128)
    y_{t+1} = h2 @ W3 + b3                                   (8)
    output ys[:, t] = y_t

Strategy (pure data parallel, batch 4096 -> 8 cores x 512):
  * feature-major layout: features on SBUF partitions, batch on the free dim.
  * y-history lives in 4 ring slots of a [128, B] staging tile, one slot per
    32-partition strip (SBUF APs must start at partition 0/32/64/96).  The
    x @ W1 product becomes: one K=128 matmul against phase-permuted W1 blocks
    (C_p, zero rows where a slot is semantically dead), one K=20 sliding
    u-window matmul, and one composed (W3 @ A0) matmul from h2 directly, so
    the recurrent cycle is just tanh -> mm(W2) -> tanh -> mm(W3 A0).
  * y_{t-4} is read from the slot y_t is about to overwrite: emission order
    (mmX before the staging write) makes Tile sequence the write after the
    read, so no extra buffering is needed.
  * outputs retire from the staging tile by raw feature-major DMA every 4
    steps; the host does the final [T,8,B] -> [B,T,8] transpose.
  * matmul operands are fp16 (1 cycle/row, fp32 PSUM accumulate); the
    5-step fading memory of the state keeps fp16 error flat (~6e-4).
"""

import numpy as np

NP_, NY, NU = 4, 8, 4
B, T, H = 4096, 256, 128
NCORES = 8
BC = B // NCORES  # 512 batch rows per core
CHUNKS = 1        # column chunks for the critical tanh/matmul cycle
CW = BC // CHUNKS
PF = 6            # u-window DMA prefetch depth (steps ahead)
NSLOT = 4         # y ring slots (one per 32-partition strip)

_COMPILED = {}


def _build_program():
    import concourse.mybir as mybir
    import concourse.tile as tile
    from concourse import bacc

    f32 = mybir.dt.float32
    fh = mybir.dt.float16
    Tanh = mybir.ActivationFunctionType.Tanh

    nc = bacc.Bacc("TRN2", target_bir_lowering=False, debug=False)

    d_stag0 = nc.dram_tensor("stag0", [128, BC], fh, kind="ExternalInput")
    d_ucon = nc.dram_tensor("ucon", [T, 128, BC], fh, kind="ExternalInput")
    # 8 C matrices: [0..3] steady phases (t % 4), [4..7] boot steps t=0..3
    d_cmats = nc.dram_tensor("cmats", [128, 8 * 128], fh, kind="ExternalInput")
    d_w2 = nc.dram_tensor("w2", [128, 128], fh, kind="ExternalInput")
    d_wc = nc.dram_tensor("wc", [128, 128], fh, kind="ExternalInput")
    d_w3 = nc.dram_tensor("w3", [128, 8], fh, kind="ExternalInput")
    d_b1 = nc.dram_tensor("b1v", [128, 1], f32, kind="ExternalInput")
    d_b1b = nc.dram_tensor("b1b", [128, 1], f32, kind="ExternalInput")
    d_b2 = nc.dram_tensor("b2v", [128, 1], f32, kind="ExternalInput")
    d_b3 = nc.dram_tensor("b3v", [8, 1], f32, kind="ExternalInput")
    d_out2 = nc.dram_tensor("out2", [T // 4, 4, 8, BC], fh, kind="ExternalOutput")

    with tile.TileContext(nc) as tc:
        with (
            tc.tile_pool(name="const", bufs=1) as cpool,
            tc.tile_pool(name="stagp", bufs=1) as spool,
            tc.tile_pool(name="upool", bufs=8) as upool,
            tc.tile_pool(name="hpool", bufs=2) as hpool,
            tc.tile_pool(name="ph1", bufs=2, space="PSUM") as ph1p,
            tc.tile_pool(name="ph2", bufs=2, space="PSUM") as ph2p,
            tc.tile_pool(name="pyp", bufs=2, space="PSUM") as pypp,
        ):
            t_cm = cpool.tile_from(d_cmats[:])
            t_w2 = cpool.tile_from(d_w2[:])
            t_wc = cpool.tile_from(d_wc[:])
            t_w3 = cpool.tile_from(d_w3[:])
            t_b1 = cpool.tile_from(d_b1[:])
            t_b1b = cpool.tile_from(d_b1b[:])
            t_b2 = cpool.tile_from(d_b2[:])
            t_b3 = cpool.tile_from(d_b3[:])

            stag = spool.tile([128, BC], fh, name="stag")
            nc.sync.dma_start(stag[:], d_stag0[:])

            def cmat(i):
                return t_cm[:, 128 * i:128 * i + 128]

            utiles = {}

            def prefetch_u(tt):
                ut = upool.tile([128, BC], fh, name="uw", tag="uw")
                nc.sync.dma_start(ut[:], d_ucon[tt])
                utiles[tt] = ut

            for tt in range(PF):
                prefetch_u(tt)

            def emit_group_xu(tt, ph1_t):
                """y-history matmul + u-contribution add for step tt."""
                cidx = 4 + tt if tt < 4 else tt % NSLOT
                nc.tensor.matmul(
                    ph1_t[:, :],
                    cmat(cidx),
                    stag[:, :],
                    start=True, stop=(tt == 0), skip_group_check=True,
                )
                # off-path DVE add of the host-precomputed u-window term
                nc.vector.tensor_add(ph1_t[:, :], ph1_t[:, :], utiles.pop(tt)[:, :])

            def flush(t0):
                """Export y_{t0..t0+3} (all 4 slots) feature-major to DRAM;
                the host transposes to batch-major at the end.

                Emitted at step t0+3 BEFORE that step's staging write, so slot
                (t0+4)%4 still holds y_{t0}.
                """
                for s in range(4):
                    nc.sync.dma_start(
                        d_out2[t0 // 4, s], stag[32 * s:32 * s + 8, :]
                    )

            ph1_cur = ph1p.tile([128, BC], f32, name="h1p", tag="h1p")
            emit_group_xu(0, ph1_cur)

            for t in range(T):
                # --- tanh1 ---
                h1_t = hpool.tile([128, BC], fh, name="h1", tag="h1")
                bias1 = t_b1b if t == 0 else t_b1
                for c in range(CHUNKS):
                    cs = slice(c * CW, (c + 1) * CW)
                    nc.scalar.activation(
                        h1_t[:, cs], ph1_cur[:, cs], Tanh, bias=bias1[:, 0:1]
                    )

                # --- mm2 ---
                ph2_t = ph2p.tile([128, BC], f32, name="h2p", tag="h2p")
                for c in range(CHUNKS):
                    cs = slice(c * CW, (c + 1) * CW)
                    nc.tensor.matmul(
                        ph2_t[:, cs],
                        t_w2[:, :],
                        h1_t[:, cs],
                    )

                # --- pre-issue next step's x-side matmuls (off critical path;
                #     must precede this step's staging write for the stale
                #     y_{t-3} read to stay dependency-free) ---
                ph1_next = None
                if t + 1 < T:
                    ph1_next = ph1p.tile([128, BC], f32, name="h1p", tag="h1p")
                    emit_group_xu(t + 1, ph1_next)

                # --- tanh2 ---
                h2_t = hpool.tile([128, BC], fh, name="h2", tag="h2")
                for c in range(CHUNKS):
                    cs = slice(c * CW, (c + 1) * CW)
                    nc.scalar.activation(
                        h2_t[:, cs], ph2_t[:, cs], Tanh, bias=t_b2[:, 0:1]
                    )

                # --- mmC: h1pre_{t+1} += (W3 A0)^T h2_t  (closes the group) ---
                if t + 1 < T:
                    for c in range(CHUNKS):
                        cs = slice(c * CW, (c + 1) * CW)
                        nc.tensor.matmul(
                            ph1_next[:, cs],
                            t_wc[:, :],
                            h2_t[:, cs],
                            start=False, stop=(c == CHUNKS - 1),
                            skip_group_check=True,
                        )

                # --- output flush (before this step's staging write) ---
                if t % 4 == 3:
                    flush(t - 3)

                # --- mm3 + staging write (y_{t+1} = W3^T h2 + b3) ---
                if t < T - 1:
                    pyp_t = pypp.tile([8, BC], f32, name="yp", tag="yp")
                    nc.tensor.matmul(
                        pyp_t[:, :], t_w3[:, :], h2_t[:, :]
                    )
                    s_new = (t + 1) % NSLOT
                    nc.vector.tensor_scalar_add(
                        stag[32 * s_new:32 * s_new + 8, :], pyp_t[:, :], t_b3[:, 0:1]
                    )

                if t + PF < T:
                    prefetch_u(t + PF)

                ph1_cur = ph1_next

    nc.compile()
    return nc


def _host_prep(useq, yz0, W1, b1, W2, b2, W3, b3):
    """Build the per-core input maps (all host-side numpy)."""
    useq = np.ascontiguousarray(useq, dtype=np.float32)
    yz0 = np.ascontiguousarray(yz0, dtype=np.float32)
    W1 = np.asarray(W1, dtype=np.float32)
    W2 = np.ascontiguousarray(W2, dtype=np.float32)
    W3 = np.ascontiguousarray(W3, dtype=np.float32)
    b1 = np.asarray(b1, dtype=np.float32)
    b2 = np.asarray(b2, dtype=np.float32)
    b3 = np.asarray(b3, dtype=np.float32)

    A = {0: W1[0:8], 4: W1[8:16], 3: W1[16:24], 2: W1[24:32], 1: W1[32:40]}
    Bstack = W1[40:60]  # u_{t-4..t} stacked chronologically

    # staging rows: slot s -> [32s, 32s+8) holds y ring;
    #               boot block s -> [32s+8, 32s+16) holds y_{-(s+1)}
    cmats = np.zeros((8, 128, 128), dtype=np.float32)
    for p in range(NSLOT):  # steady phases, t >= 4: every slot one A_k
        for s in range(NSLOT):
            k = ((p - s - 1) % 4) + 1
            cmats[p, 32 * s:32 * s + 8] = A[k]
    for tt in range(4):  # boot steps t=0..3
        cb = cmats[4 + tt]
        for k in range(1, 5):
            if tt - k >= 0:
                s = (tt - k) % 4
                cb[32 * s:32 * s + 8] += A[k]
            else:
                s = k - tt - 1
                cb[32 * s + 8:32 * s + 16] += A[k]
        if tt == 0:
            cb[0:8] += A[0]  # slot 0 carries y_0 directly at t=0
    cmats2d = np.ascontiguousarray(
        cmats.transpose(1, 0, 2).reshape(128, 8 * 128)
    )

    WC = np.ascontiguousarray(W3 @ A[0])          # [128, 128]
    b1_eff = (b1 + A[0].T @ b3).reshape(128, 1)   # mmC path lacks A0^T b3
    b1_boot = b1.reshape(128, 1)
    b2v = b2.reshape(128, 1)
    b3v = b3.reshape(8, 1)
    ident = np.eye(128, dtype=np.float16)

    in_maps = []
    for c in range(NCORES):
        bs = slice(c * BC, (c + 1) * BC)
        u_c = useq[bs]      # [BC, T, 4]
        yz_c = yz0[bs]      # [BC, 56]

        stag0 = np.zeros((128, BC), dtype=np.float32)
        stag0[0:8] = yz_c[:, 0:8].T               # slot 0 = y_0
        for s in range(4):                         # boot blocks y_{-(s+1)}
            blk = yz_c[:, 8 + 8 * (3 - s):16 + 8 * (3 - s)]  # ypseq newest last
            stag0[32 * s + 8:32 * s + 16] = blk.T

        # sliding u-windows for the K=20 u matmul
        uhist = yz_c[:, 40:56].reshape(BC, 4, 4)          # u_{-4..-1}
        uext = np.concatenate([uhist, u_c], axis=1)       # [BC, T+4, 4]
        sw = np.lib.stride_tricks.sliding_window_view(uext, 5, axis=1)
        # sw: [BC, T, 4, 5] -> uwin [T, 20, BC] (chronological rows)
        uwin = np.ascontiguousarray(sw.transpose(1, 3, 2, 0).reshape(T, 20, BC))
        ucon = np.matmul(Bstack.T[None].astype(np.float32), uwin)  # [T,128,BC]

        in_maps.append({
            "stag0": stag0.astype(np.float16),
            "ucon": ucon.astype(np.float16),
            "cmats": cmats2d.astype(np.float16),
            "w2": W2.astype(np.float16),
            "wc": WC.astype(np.float16),
            "w3": W3.astype(np.float16),
            "b1v": np.ascontiguousarray(b1_eff),
            "b1b": np.ascontiguousarray(b1_boot),
            "b2v": np.ascontiguousarray(b2v),
            "b3v": np.ascontiguousarray(b3v),
        })
    return in_maps


def get_program():
    if "nc" not in _COMPILED:
        _COMPILED["nc"] = _build_program()
    return _COMPILED["nc"]


def _enable_ldw_opt():
    """Allow walrus to double-buffer LDWEIGHTS (background weight loads).

    The environment default is --enable-ldw-opt=false, which serializes
    every LDWEIGHTS behind the previous matmul's drain; with ~6 weight
    switches per RNN step that costs ~2x on the tensor engine.
    """
    try:
        from concourse.compiler_utils import get_compiler_flags, set_compiler_flags

        flags = get_compiler_flags()
        new = [f.replace("--enable-ldw-opt=false", "--enable-ldw-opt=true") for f in flags]
        if new != flags:
            set_compiler_flags(new)
    except Exception:
        pass


def run_cores(in_maps, **kwargs):
    from concourse.bass_utils import run_bass_kernel_spmd

    _enable_ldw_opt()
    nc = get_program()
    return run_bass_kernel_spmd(nc, in_maps, core_ids=list(range(NCORES)), **kwargs)


def assemble(res):
    outs = []
    for r in res.results:
        buf = np.asarray(r["out2"], dtype=np.float32)   # [T/4, 4, 8, BC]
        ys = buf.transpose(3, 0, 1, 2).reshape(BC, T, NY)
        outs.append(ys)
    return np.concatenate(outs, axis=0)


def kernel(useq, yz0, W1, b1, W2, b2, W3, b3):
    in_maps = _host_prep(useq, yz0, W1, b1, W2, b2, W3, b3)
    res = run_cores(in_maps)
    return assemble(res)

